# revision 3
# baseline (speedup 1.0000x reference)
"""Cached multi-head attention on 8 TRN2 NeuronCores.

Sharding: core c = 2*b + g handles batch b (of 4) and head-group g (of 2,
8 heads each) -- data parallel on batch x tensor parallel on heads.
Column-parallel Wq/Wk/Wv, row-parallel Wo; the Wo all-reduce (sum of the
two head-group partials per batch) is done on host during the unshard,
along with the bo bias add.

Device layout (per core), all matmuls in float32r (full PE rate):
  xT = x.T in HBM (host pre-transposed). Projections:
    qT[d,t] = sum_c WqT[c,d] xqT[c,t]  (+bq)   -> SBUF pair tiles [128, T]
    kT likewise; v[s,d] = sum_c xvT[c,s] WvT[c,d] (+bv via K=1 ones matmul)
  Attention per head-pair (2 heads row-packed in the 128-partition dim):
    ST[s,t] = kT.T @ qT   (K=64 row-tiled, both heads concurrent)
    P = exp(ST/8)         (ScalarE, free scale; no max-subtract needed --
                           scores are O(1) by construction)
    oT_aug = [V|1].T @ P  (K=128, M=65; row 64 = softmax denominators)
    o = oT * (1/denom)    (DVE mult with gpsimd-broadcast reciprocal)
  Out-projection: out[t,e] = sum_d oT[d,t] WoT[d,e], accumulated over the
  4 pair-chunks of d.

Causal masks get a fast path: blocks above the diagonal are skipped,
diagonal blocks use shortened matmuls + gpsimd affine_select zeroing.
Arbitrary masks fall back to per-block skip/plain/mixed classification
with host-shipped multiplicative mask tiles.
"""

import math
import numpy as np

import concourse.bass as bass
import concourse.mybir as mybir
import concourse.tile as tile
from concourse import bacc
from concourse.bass_utils import run_bass_kernel_spmd

F32 = mybir.dt.float32
F32R = mybir.dt.float32r
AF = mybir.ActivationFunctionType
ts = bass.ts

B, T, D, H = 4, 2048, 1024, 16
HD = D // H          # 64
NCORE = 8
DG = D // 2          # 512 dims per core (8 heads)
NPAIR = 4            # head pairs per core
SB = 128             # s-block size
TC = 512             # attention t-chunk
NTC = T // TC        # 4
NSB = T // SB        # 16
PC = 256             # projection t-chunk (x streaming granularity)
NPC = T // PC        # 8
CCH = D // 128       # 8 contraction chunks

_cache = {}
last_result = {}


def _classify_blocks(mask):
    """Per (s_blk, t_chunk) classification, unioned across batches (SPMD).

    Returns (mode, cls, mixed_list) where cls[s][i] in {0 skip, 1 plain,
    2 mixed} and mixed_list orders the mixed blocks.
    """
    causal = np.triu(np.ones((T, T), dtype=bool), k=1)
    if all(np.array_equal(mask[b], causal) for b in range(B)):
        return "causal", None, None
    cls = np.zeros((NSB, NTC), dtype=np.int64)
    for s in range(NSB):
        for i in range(NTC):
            blk = mask[:, i * TC:(i + 1) * TC, s * SB:(s + 1) * SB]  # [B,t,s]
            if blk.any():
                cls[s, i] = 2 if not blk.all() else 0
            else:
                cls[s, i] = 1
    # a block masked in every batch can still differ per batch -> recheck:
    # skip only if all batches fully masked; mixed if some batch partially
    # or batches disagree (all-masked vs all-valid across batches)
    for s in range(NSB):
        for i in range(NTC):
            blk = mask[:, i * TC:(i + 1) * TC, s * SB:(s + 1) * SB]
            per_b_all = [mask[b, i * TC:(i + 1) * TC, s * SB:(s + 1) * SB].all()
                         for b in range(B)]
            per_b_any = [mask[b, i * TC:(i + 1) * TC, s * SB:(s + 1) * SB].any()
                         for b in range(B)]
            if all(per_b_all):
                cls[s, i] = 0
            elif not any(per_b_any):
                cls[s, i] = 1
            else:
                cls[s, i] = 2
    mixed = [(s, i) for s in range(NSB) for i in range(NTC) if cls[s, i] == 2]
    return "general", cls, mixed


def _build(mode, cls, n_mixed):
    nc = bacc.Bacc("TRN2", target_bir_lowering=False, debug=False,
                   num_devices=NCORE)
    d = {}
    for nm in ("xq", "xk", "xv"):
        d[nm] = nc.dram_tensor(nm, [D, T], F32R, kind="ExternalInput").ap()
    for nm in ("wq", "wk", "wv"):
        d[nm] = nc.dram_tensor(nm, [D, DG], F32R, kind="ExternalInput").ap()
    d["wo"] = nc.dram_tensor("wo", [DG, D], F32R, kind="ExternalInput").ap()
    d["bq"] = nc.dram_tensor("bq", [128, NPAIR], F32, kind="ExternalInput").ap()
    d["bk"] = nc.dram_tensor("bk", [128, NPAIR], F32, kind="ExternalInput").ap()
    d["bv"] = nc.dram_tensor("bv", [1, DG], F32R, kind="ExternalInput").ap()
    d["ones1"] = nc.dram_tensor("ones1", [1, 128], F32R, kind="ExternalInput").ap()
    d["onesv"] = nc.dram_tensor("onesv", [128, H // 2], F32R,
                                kind="ExternalInput").ap()
    if n_mixed:
        d["mmask"] = nc.dram_tensor("mmask", [n_mixed, SB, TC], F32R,
                                    kind="ExternalInput").ap()
    out_d = nc.dram_tensor("out", [T, D], F32, kind="ExternalOutput").ap()

    with tile.TileContext(nc) as tc:
        with (
            tc.tile_pool(name="persist", bufs=1) as pp,
            tc.tile_pool(name="ps512", bufs=5, space="PSUM") as ps5,
            tc.tile_pool(name="psot", bufs=3, space="PSUM") as pso,
        ):
            # ---- persistent tiles ---------------------------------------
            bq_sb = pp.tile([128, NPAIR], F32, tag="bq")
            bk_sb = pp.tile([128, NPAIR], F32, tag="bk")
            bv_sb = pp.tile([1, DG], F32R, tag="bv")
            ones1_sb = pp.tile([1, 128], F32R, tag="ones1")
            nc.sync.dma_start(out=bq_sb[:], in_=d["bq"][:])
            nc.sync.dma_start(out=bk_sb[:], in_=d["bk"][:])
            nc.sync.dma_start(out=bv_sb[:], in_=d["bv"][:])
            nc.sync.dma_start(out=ones1_sb[:], in_=d["ones1"][:])

            qT = [pp.tile([128, T], F32R, tag=f"qT{p}", name=f"qT{p}") for p in range(NPAIR)]
            kT = [pp.tile([128, T], F32R, tag=f"kT{p}", name=f"kT{p}") for p in range(NPAIR)]
            oT = [pp.tile([128, T], F32R, tag=f"oT{p}", name=f"oT{p}") for p in range(NPAIR)]
            HV = HD + 1  # 65: V columns + ones column per head
            v_sb = [pp.tile([128, 8 * HV], F32R, tag=f"v{s}", name=f"v{s}") for s in range(NSB)]
            for s in range(NSB):
                ones_cols = v_sb[s][:].rearrange("p (h c) -> p h c", c=HV)[:, :, HD:HV]
                nc.sync.dma_start(out=ones_cols, in_=d["onesv"][:])

            # ---- phase 1: projections (one sub-pool per tensor) ---------
            for nm, dst, bias in (("q", qT, bq_sb), ("k", kT, bk_sb)):
                with tc.tile_pool(name=f"proj_{nm}", bufs=2) as sp:
                    w = sp.tile([128, CCH * DG], F32R, tag="w", bufs=1, name=f"w{nm}")
                    for c in range(CCH):
                        nc.sync.dma_start(out=w[:, ts(c, DG)],
                                          in_=d["w" + nm][ts(c, 128), :])
                    for tau in range(NPC):
                        x = sp.tile([128, CCH * PC], F32R, tag="x", name=f"x{nm}")
                        for c in range(CCH):
                            nc.sync.dma_start(out=x[:, ts(c, PC)],
                                              in_=d["x" + nm][ts(c, 128), ts(tau, PC)])
                        for p in range(NPAIR):
                            ps = ps5.tile([128, TC], F32, tag="b512")
                            for c in range(CCH):
                                nc.tensor.matmul(
                                    ps[:, 0:PC],
                                    w[:, c * DG + p * 128:c * DG + (p + 1) * 128],
                                    x[:, ts(c, PC)],
                                    start=(c == 0), stop=(c == CCH - 1))
                            nc.vector.tensor_scalar(
                                out=dst[p][:, ts(tau, PC)], in0=ps[:, 0:PC],
                                scalar1=bias[:, p:p + 1], scalar2=None,
                                op0=mybir.AluOpType.add)
            with tc.tile_pool(name="proj_v", bufs=2) as sp:
                w = sp.tile([128, CCH * DG], F32R, tag="w", bufs=1, name="wv_sb")
                for c in range(CCH):
                    nc.sync.dma_start(out=w[:, ts(c, DG)], in_=d["wv"][ts(c, 128), :])
                for tau in range(NPC):
                    x = sp.tile([128, CCH * PC], F32R, tag="x", name="xv_sb")
                    for c in range(CCH):
                        nc.sync.dma_start(out=x[:, ts(c, PC)],
                                          in_=d["xv"][ts(c, 128), ts(tau, PC)])
                    for u in range(PC // SB):
                        sigma = tau * (PC // SB) + u
                        ps = ps5.tile([128, TC], F32, tag="b512")
                        for c in range(CCH):
                            nc.tensor.matmul(
                                ps[:],
                                x[:, c * PC + u * SB:c * PC + (u + 1) * SB],
                                w[:, ts(c, DG)],
                                start=(c == 0), stop=False)
                        nc.tensor.matmul(ps[:], ones1_sb[:], bv_sb[:],
                                         start=False, stop=True)
                        vdst = v_sb[sigma][:].rearrange("p (h c) -> p h c", c=HV)[:, :, 0:HD]
                        vsrc = ps[:].rearrange("p (h c) -> p h c", c=HD)
                        nc.vector.tensor_copy(vdst, vsrc)

            # ---- phase 2: attention -------------------------------------
            scale = 1.0 / math.sqrt(HD)
            with (
                tc.tile_pool(name="attn", bufs=2) as sp,
                tc.tile_pool(name="small", bufs=2) as mp,
            ):
                for p in range(NPAIR):
                    for i in range(NTC):
                        otA = pso.tile([HV, TC], F32, tag="ot")
                        otB = pso.tile([HV, TC], F32, tag="ot")
                        if mode == "causal":
                            blocks = []
                            for s_blk in range(4 * i + 4):
                                j = s_blk - 4 * i
                                if j < 0:
                                    blocks.append((s_blk, i * TC, TC, False))
                                else:
                                    s0 = SB * s_blk
                                    toff = s0 if j < 3 else s0 - SB
                                    blocks.append((s_blk, toff, TC * (i + 1) - toff, True))
                        else:
                            blocks = [(s_blk, i * TC, TC, False)
                                      for s_blk in range(NSB) if cls[s_blk, i] != 0]
                        started = False
                        for s_blk, toff, n, diag in blocks:
                            s0 = SB * s_blk
                            stA = ps5.tile([128, TC], F32, tag="b512")
                            stB = ps5.tile([128, TC], F32, tag="b512")
                            nc.tensor.matmul(
                                stA[:, 0:n], kT[p][0:HD, ts(s_blk, SB)],
                                qT[p][0:HD, toff:toff + n],
                                start=True, stop=True, tile_position=(0, 0))
                            nc.tensor.matmul(
                                stB[:, 0:n], kT[p][HD:128, ts(s_blk, SB)],
                                qT[p][HD:128, toff:toff + n],
                                start=True, stop=True, tile_position=(64, 0))
                            pA = sp.tile([128, TC], F32R, tag="pA")
                            pB = sp.tile([128, TC], F32R, tag="pB")
                            nc.scalar.activation(pA[:, 0:n], stA[:, 0:n], AF.Exp,
                                                 scale=scale)
                            nc.scalar.activation(pB[:, 0:n], stB[:, 0:n], AF.Exp,
                                                 scale=scale)
                            if mode == "causal" and diag:
                                w_ = s0 + SB - toff
                                for ptile in (pA, pB):
                                    nc.gpsimd.affine_select(
                                        out=ptile[:, 0:w_], in_=ptile[:, 0:w_],
                                        compare_op=mybir.AluOpType.is_ge,
                                        fill=0.0, base=toff - s0,
                                        pattern=[[1, w_]], channel_multiplier=-1)
                            elif mode == "general" and cls[s_blk, i] == 2:
                                mmt = sp.tile([SB, TC], F32R, tag="mmask")
                                nc.sync.dma_start(out=mmt[:],
                                                  in_=d["mmask"][mixed_idx[(s_blk, i)]])
                                nc.vector.tensor_mul(pA[:, 0:n], pA[:, 0:n], mmt[:, 0:n])
                                nc.vector.tensor_mul(pB[:, 0:n], pB[:, 0:n], mmt[:, 0:n])
                            co = toff - i * TC
                            vv = v_sb[s_blk][:].rearrange("p (h c) -> p h c", c=HV)
                            nc.tensor.matmul(otA[:, co:co + n], vv[:, 2 * p, :],
                                             pA[:, 0:n], start=not started,
                                             stop=(s_blk == blocks[-1][0]))
                            nc.tensor.matmul(otB[:, co:co + n], vv[:, 2 * p + 1, :],
                                             pB[:, 0:n], start=not started,
                                             stop=(s_blk == blocks[-1][0]))
                            started = True
                        # epilogue: reciprocal of denominators, broadcast, scale
                        for hh, ot_ps in ((0, otA), (1, otB)):
                            rr = mp.tile([HV, TC], F32, tag="rr")
                            nc.vector.reciprocal(rr[HD:HV, :], ot_ps[HD:HV, :])
                            r1 = mp.tile([1, TC], F32, tag="r1")
                            nc.sync.dma_start(out=r1[:], in_=rr[HD:HV, :])
                            rb = mp.tile([HD, TC], F32, tag="rb")
                            nc.gpsimd.partition_broadcast(rb[:], r1[:])
                            if hh == 0:
                                nc.vector.tensor_mul(oT[p][0:HD, ts(i, TC)],
                                                     ot_ps[0:HD, :], rb[:])
                            else:
                                stg = mp.tile([HD, TC], F32R, tag="stg")
                                nc.vector.tensor_mul(stg[:], ot_ps[0:HD, :], rb[:])
                                nc.sync.dma_start(out=oT[p][HD:128, ts(i, TC)],
                                                  in_=stg[:])

            # ---- phase 3: output projection ------------------------------
            with tc.tile_pool(name="outp", bufs=2) as sp:
                wo_sb = sp.tile([128, NPAIR * D], F32R, tag="wo", bufs=1)
                for p in range(NPAIR):
                    nc.sync.dma_start(out=wo_sb[:, ts(p, D)],
                                      in_=d["wo"][ts(p, 128), :])
                for tt in range(T // 128):
                    ob = sp.tile([128, D], F32, tag="ob")
                    for e in range(2):
                        ps = ps5.tile([128, TC], F32, tag="b512")
                        for p in range(NPAIR):
                            nc.tensor.matmul(
                                ps[:], oT[p][:, ts(tt, 128)],
                                wo_sb[:, p * D + e * TC:p * D + (e + 1) * TC],
                                start=(p == 0), stop=(p == NPAIR - 1))
                        nc.vector.tensor_copy(ob[:, ts(e, TC)], ps[:])
                    nc.sync.dma_start(out=out_d[ts(tt, 128), :], in_=ob[:])

    nc.compile()
    return nc


def kernel(**inputs):
    query = np.asarray(inputs["query"], np.float32)
    key = np.asarray(inputs["key"], np.float32)
    value = np.asarray(inputs["value"], np.float32)
    mask = np.asarray(inputs["mask"], bool)
    Wq, bq = np.asarray(inputs["Wq"], np.float32), np.asarray(inputs["bq"], np.float32)
    Wk, bk = np.asarray(inputs["Wk"], np.float32), np.asarray(inputs["bk"], np.float32)
    Wv, bv = np.asarray(inputs["Wv"], np.float32), np.asarray(inputs["bv"], np.float32)
    Wo, bo = np.asarray(inputs["Wo"], np.float32), np.asarray(inputs["bo"], np.float32)

    mode, cls, mixed = _classify_blocks(mask)
    global mixed_idx
    if mode == "general":
        mixed_idx = {blk: n for n, blk in enumerate(mixed)}
        n_mixed = len(mixed)
    else:
        mixed_idx, n_mixed = {}, 0

    key_sig = (mode, tuple(cls.ravel()) if cls is not None else None)
    if key_sig not in _cache:
        _cache[key_sig] = _build(mode, cls, n_mixed)
    nc = _cache[key_sig]

    in_maps = []
    xT = {}
    for b in range(B):
        xT[("xq", b)] = np.ascontiguousarray(query[b].T)
        xT[("xk", b)] = np.ascontiguousarray(key[b].T)
        xT[("xv", b)] = np.ascontiguousarray(value[b].T)
    for core in range(NCORE):
        b, g = core // 2, core % 2
        sl = slice(g * DG, (g + 1) * DG)
        im = {
            "xq": xT[("xq", b)], "xk": xT[("xk", b)], "xv": xT[("xv", b)],
            "wq": np.ascontiguousarray(Wq[sl, :].T),
            "wk": np.ascontiguousarray(Wk[sl, :].T),
            "wv": np.ascontiguousarray(Wv[sl, :].T),
            "wo": np.ascontiguousarray(Wo[:, sl].T),
            "bq": np.ascontiguousarray(bq[sl].reshape(NPAIR, 128).T),
            "bk": np.ascontiguousarray(bk[sl].reshape(NPAIR, 128).T),
            "bv": np.ascontiguousarray(bv[sl])[None, :],
            "ones1": np.ones((1, 128), np.float32),
            "onesv": np.ones((128, H // 2), np.float32),
        }
        if n_mixed:
            mm = np.empty((n_mixed, SB, TC), np.float32)
            for n, (s_blk, i) in enumerate(mixed):
                blk = mask[b, i * TC:(i + 1) * TC, s_blk * SB:(s_blk + 1) * SB]
                mm[n] = (~blk.T).astype(np.float32)
            im["mmask"] = mm
        in_maps.append(im)

    r = run_bass_kernel_spmd(nc, in_maps, core_ids=list(range(NCORE)))
    last_result["exec_time_ns"] = r.exec_time_ns
    out = np.empty((B, T, D), np.float32)
    for b in range(B):
        out[b] = r.results[2 * b]["out"] + r.results[2 * b + 1]["out"]
    out += bo[None, None, :]
    return out


# revision 5
# speedup vs baseline: 1.4302x; 1.4302x over previous
"""Cached multi-head attention on 8 TRN2 NeuronCores.

Sharding: core c = 2*b + g handles batch b (of 4) and head-group g (of 2,
8 heads each) -- data parallel on batch x tensor parallel on heads.
Column-parallel Wq/Wk/Wv, row-parallel Wo; the Wo all-reduce (sum of the
two head-group partials per batch) is done on host during the unshard,
along with the bo bias add.

Device layout (per core), all matmuls in float32r (full PE rate):
  xT = x.T in HBM (host pre-transposed). Projections:
    qT[d,t] = sum_c WqT[c,d] xqT[c,t]  (+bq)   -> SBUF pair tiles [128, T]
    kT likewise; v[s,d] = sum_c xvT[c,s] WvT[c,d] (+bv via K=1 ones matmul)
  Attention per head-pair (2 heads row-packed in the 128-partition dim):
    ST[s,t] = kT.T @ qT   (K=64 row-tiled, both heads concurrent)
    P = exp(ST/8)         (ScalarE, free scale; no max-subtract needed --
                           scores are O(1) by construction)
    oT_aug = [V|1].T @ P  (K=128, M=65; row 64 = softmax denominators)
    o = oT * (1/denom)    (DVE mult with gpsimd-broadcast reciprocal)
  Out-projection: out[t,e] = sum_d oT[d,t] WoT[d,e], accumulated over the
  4 pair-chunks of d.

Causal masks get a fast path: blocks above the diagonal are skipped,
diagonal blocks use shortened matmuls + gpsimd affine_select zeroing.
Arbitrary masks fall back to per-block skip/plain/mixed classification
with host-shipped multiplicative mask tiles.
"""

import math
import ml_dtypes
import numpy as np

import concourse.bass as bass
import concourse.mybir as mybir
import concourse.tile as tile
from concourse import bacc
from concourse.bass_utils import run_bass_kernel_spmd

F32 = mybir.dt.float32
F32R = mybir.dt.float32r
BF16 = mybir.dt.bfloat16
AF = mybir.ActivationFunctionType
ts = bass.ts

B, T, D, H = 4, 2048, 1024, 16
HD = D // H          # 64
NCORE = 8
DG = D // 2          # 512 dims per core (8 heads)
NPAIR = 4            # head pairs per core
SB = 128             # s-block size
TC = 512             # attention t-chunk
NTC = T // TC        # 4
NSB = T // SB        # 16
PC = 256             # projection t-chunk (x streaming granularity)
NPC = T // PC        # 8
CCH = D // 128       # 8 contraction chunks

_cache = {}
last_result = {}


def _classify_blocks(mask):
    """Per (s_blk, t_chunk) classification, unioned across batches (SPMD).

    Returns (mode, cls, mixed_list) where cls[s][i] in {0 skip, 1 plain,
    2 mixed} and mixed_list orders the mixed blocks.
    """
    causal = np.triu(np.ones((T, T), dtype=bool), k=1)
    if all(np.array_equal(mask[b], causal) for b in range(B)):
        return "causal", None, None
    cls = np.zeros((NSB, NTC), dtype=np.int64)
    for s in range(NSB):
        for i in range(NTC):
            blk = mask[:, i * TC:(i + 1) * TC, s * SB:(s + 1) * SB]  # [B,t,s]
            if blk.any():
                cls[s, i] = 2 if not blk.all() else 0
            else:
                cls[s, i] = 1
    # a block masked in every batch can still differ per batch -> recheck:
    # skip only if all batches fully masked; mixed if some batch partially
    # or batches disagree (all-masked vs all-valid across batches)
    for s in range(NSB):
        for i in range(NTC):
            blk = mask[:, i * TC:(i + 1) * TC, s * SB:(s + 1) * SB]
            per_b_all = [mask[b, i * TC:(i + 1) * TC, s * SB:(s + 1) * SB].all()
                         for b in range(B)]
            per_b_any = [mask[b, i * TC:(i + 1) * TC, s * SB:(s + 1) * SB].any()
                         for b in range(B)]
            if all(per_b_all):
                cls[s, i] = 0
            elif not any(per_b_any):
                cls[s, i] = 1
            else:
                cls[s, i] = 2
    mixed = [(s, i) for s in range(NSB) for i in range(NTC) if cls[s, i] == 2]
    return "general", cls, mixed


def _build(mode, cls, n_mixed):
    nc = bacc.Bacc("TRN2", target_bir_lowering=False, debug=False,
                   num_devices=NCORE)
    d = {}
    for nm in ("xq", "xk", "xv"):
        d[nm] = nc.dram_tensor(nm, [D, T], F32R, kind="ExternalInput").ap()
    for nm in ("wq", "wk", "wv"):
        d[nm] = nc.dram_tensor(nm, [D, DG], F32R, kind="ExternalInput").ap()
    d["wo"] = nc.dram_tensor("wo", [DG, D], BF16, kind="ExternalInput").ap()
    d["bq"] = nc.dram_tensor("bq", [128, NPAIR], F32, kind="ExternalInput").ap()
    d["bk"] = nc.dram_tensor("bk", [128, NPAIR], F32, kind="ExternalInput").ap()
    d["bv"] = nc.dram_tensor("bv", [1, DG], F32R, kind="ExternalInput").ap()
    d["ones1"] = nc.dram_tensor("ones1", [1, 128], F32R, kind="ExternalInput").ap()
    d["onesv"] = nc.dram_tensor("onesv", [128, H // 2], BF16,
                                kind="ExternalInput").ap()
    if n_mixed:
        d["mmask"] = nc.dram_tensor("mmask", [n_mixed, SB, TC], BF16,
                                    kind="ExternalInput").ap()
    out_d = nc.dram_tensor("out", [T, D], F32, kind="ExternalOutput").ap()

    with tile.TileContext(nc) as tc:
        with (
            tc.tile_pool(name="persist", bufs=1) as pp,
            tc.tile_pool(name="stream", bufs=2) as sp,
            tc.tile_pool(name="small", bufs=2) as mp,
            tc.tile_pool(name="psum", bufs=4, space="PSUM") as psp,
        ):
            # ---- persistent tiles & weight loads ------------------------
            w_sb = {}
            for nm in ("wq", "wk", "wv"):
                w = pp.tile([128, CCH * DG], F32R, tag=nm, name=nm + "_sb")
                for c in range(CCH):
                    nc.sync.dma_start(out=w[:, ts(c, DG)], in_=d[nm][ts(c, 128), :])
                w_sb[nm] = w
            wo_sb = pp.tile([128, NPAIR * D], BF16, tag="wo")
            for p in range(NPAIR):
                nc.sync.dma_start(out=wo_sb[:, ts(p, D)], in_=d["wo"][ts(p, 128), :])
            bq_sb = pp.tile([128, NPAIR], F32, tag="bq")
            bk_sb = pp.tile([128, NPAIR], F32, tag="bk")
            bv_sb = pp.tile([1, DG], F32R, tag="bv")
            ones1_sb = pp.tile([1, 128], F32R, tag="ones1")
            nc.sync.dma_start(out=bq_sb[:], in_=d["bq"][:])
            nc.sync.dma_start(out=bk_sb[:], in_=d["bk"][:])
            nc.sync.dma_start(out=bv_sb[:], in_=d["bv"][:])
            nc.sync.dma_start(out=ones1_sb[:], in_=d["ones1"][:])

            qT = [pp.tile([128, T], BF16, tag=f"qT{p}", name=f"qT{p}") for p in range(NPAIR)]
            kT = [pp.tile([128, T], BF16, tag=f"kT{p}", name=f"kT{p}") for p in range(NPAIR)]
            oT = [pp.tile([128, T], BF16, tag=f"oT{p}", name=f"oT{p}") for p in range(NPAIR)]
            HV = HD + 1  # 65: V columns + ones column per head
            v_sb = [pp.tile([128, 8 * HV], BF16, tag=f"v{s}", name=f"v{s}") for s in range(NSB)]
            for s in range(NSB):
                ones_cols = v_sb[s][:].rearrange("p (h c) -> p h c", c=HV)[:, :, HD:HV]
                nc.sync.dma_start(out=ones_cols, in_=d["onesv"][:])

            # ---- phase 1a: V projection ---------------------------------
            for tau in range(NPC):
                x = sp.tile([128, CCH * PC], F32R, tag="x", bufs=4, name="xv_t")
                for c in range(CCH):
                    nc.sync.dma_start(out=x[:, ts(c, PC)],
                                      in_=d["xv"][ts(c, 128), ts(tau, PC)])
                for u in range(PC // SB):
                    sigma = tau * (PC // SB) + u
                    ps = psp.tile([128, TC], F32, tag="b512")
                    for c in range(CCH):
                        nc.tensor.matmul(
                            ps[:],
                            x[:, c * PC + u * SB:c * PC + (u + 1) * SB],
                            w_sb["wv"][:, ts(c, DG)],
                            start=(c == 0), stop=False)
                    nc.tensor.matmul(ps[:], ones1_sb[:], bv_sb[:],
                                     start=False, stop=True)
                    vdst = v_sb[sigma][:].rearrange("p (h c) -> p h c", c=HV)[:, :, 0:HD]
                    vsrc = ps[:].rearrange("p (h c) -> p h c", c=HD)
                    nc.vector.tensor_copy(vdst, vsrc)

            # ---- phase 1b: Q and K projections --------------------------
            for tau in range(NPC):
                xq = sp.tile([128, CCH * PC], F32R, tag="x", bufs=4, name="xq_t")
                xk = sp.tile([128, CCH * PC], F32R, tag="x", bufs=4, name="xk_t")
                for c in range(CCH):
                    nc.sync.dma_start(out=xq[:, ts(c, PC)],
                                      in_=d["xq"][ts(c, 128), ts(tau, PC)])
                    nc.sync.dma_start(out=xk[:, ts(c, PC)],
                                      in_=d["xk"][ts(c, 128), ts(tau, PC)])
                for p in range(NPAIR):
                    for nm, xx, dst, bias in (("q", xq, qT, bq_sb),
                                              ("k", xk, kT, bk_sb)):
                        ps = psp.tile([128, TC], F32, tag="b512")
                        for c in range(CCH):
                            nc.tensor.matmul(
                                ps[:, 0:PC],
                                w_sb["w" + nm][:, c * DG + p * 128:c * DG + (p + 1) * 128],
                                xx[:, ts(c, PC)],
                                start=(c == 0), stop=(c == CCH - 1))
                        nc.vector.tensor_scalar(
                            out=dst[p][:, ts(tau, PC)], in0=ps[:, 0:PC],
                            scalar1=bias[:, p:p + 1], scalar2=None,
                            op0=mybir.AluOpType.add)

            # ---- phase 2: attention -------------------------------------
            scale = 1.0 / math.sqrt(HD)
            for p in range(NPAIR):
                for i in range(NTC):
                    otA = psp.tile([HV, TC], F32, tag="b512")
                    otB = psp.tile([HV, TC], F32, tag="b512")
                    if mode == "causal":
                        blocks = []
                        for s_blk in range(4 * i + 4):
                            j = s_blk - 4 * i
                            if j < 0:
                                blocks.append((s_blk, i * TC, TC, False))
                            else:
                                s0 = SB * s_blk
                                toff = s0 if j < 3 else s0 - SB
                                blocks.append((s_blk, toff, TC * (i + 1) - toff, True))
                    else:
                        blocks = [(s_blk, i * TC, TC, False)
                                  for s_blk in range(NSB) if cls[s_blk, i] != 0]
                    started = False
                    for s_blk, toff, n, diag in blocks:
                        s0 = SB * s_blk
                        st2 = psp.tile([128, 2 * TC], F32, tag="stAB", bufs=2)
                        nc.tensor.matmul(
                            st2[:, 0:n], kT[p][0:HD, ts(s_blk, SB)],
                            qT[p][0:HD, toff:toff + n],
                            start=True, stop=True, tile_position=(0, 0))
                        nc.tensor.matmul(
                            st2[:, TC:TC + n], kT[p][HD:128, ts(s_blk, SB)],
                            qT[p][HD:128, toff:toff + n],
                            start=True, stop=True, tile_position=(64, 0))
                        p2 = sp.tile([128, 2 * TC], BF16, tag="pAB", bufs=4)
                        if n == TC:
                            nc.scalar.activation(p2[:], st2[:], AF.Exp, scale=scale)
                        else:
                            st3 = st2[:].rearrange("p (b c) -> p b c", b=2)[:, :, 0:n]
                            p3 = p2[:].rearrange("p (b c) -> p b c", b=2)[:, :, 0:n]
                            nc.scalar.activation(p3, st3, AF.Exp, scale=scale)
                        if mode == "causal" and diag:
                            w_ = s0 + SB - toff
                            for off in (0, TC):
                                nc.gpsimd.affine_select(
                                    out=p2[:, off:off + w_], in_=p2[:, off:off + w_],
                                    compare_op=mybir.AluOpType.is_ge,
                                    fill=0.0, base=toff - s0,
                                    pattern=[[1, w_]], channel_multiplier=-1)
                        elif mode == "general" and cls[s_blk, i] == 2:
                            mmt = sp.tile([SB, TC], BF16, tag="mmask")
                            nc.sync.dma_start(out=mmt[:],
                                              in_=d["mmask"][mixed_idx[(s_blk, i)]])
                            for off in (0, TC):
                                nc.vector.tensor_mul(p2[:, off:off + n],
                                                     p2[:, off:off + n], mmt[:, 0:n])
                        co = toff - i * TC
                        vv = v_sb[s_blk][:].rearrange("p (h c) -> p h c", c=HV)
                        nc.tensor.matmul(otA[:, co:co + n], vv[:, 2 * p, :],
                                         p2[:, 0:n], start=not started,
                                         stop=(s_blk == blocks[-1][0]))
                        nc.tensor.matmul(otB[:, co:co + n], vv[:, 2 * p + 1, :],
                                         p2[:, TC:TC + n], start=not started,
                                         stop=(s_blk == blocks[-1][0]))
                        started = True
                    # epilogue: denominators -> partition 0 -> broadcast
                    # -> fast reciprocal -> scale rows
                    for hh, ot_ps in ((0, otA), (1, otB)):
                        den = mp.tile([HV, TC], F32, tag="den")
                        nc.scalar.copy(den[HD:HV, :], ot_ps[HD:HV, :])
                        r1 = mp.tile([1, TC], F32, tag="r1")
                        nc.sync.dma_start(out=r1[:], in_=den[HD:HV, :])
                        rbd = mp.tile([HD, TC], F32, tag="rbd")
                        nc.gpsimd.partition_broadcast(rbd[:], r1[:])
                        rb = mp.tile([HD, TC], F32, tag="rb")
                        nc.vector.reciprocal_approx_fast(out=rb[:], in_=rbd[:])
                        if hh == 0:
                            nc.vector.tensor_mul(oT[p][0:HD, ts(i, TC)],
                                                 ot_ps[0:HD, :], rb[:])
                        else:
                            stg = mp.tile([HD, TC], BF16, tag="stg")
                            nc.vector.tensor_mul(stg[:], ot_ps[0:HD, :], rb[:])
                            nc.sync.dma_start(out=oT[p][HD:128, ts(i, TC)],
                                              in_=stg[:])

            # ---- phase 3: output projection ------------------------------
            for tt in range(T // 128):
                ob = sp.tile([128, D], F32, tag="ob")
                for e in range(2):
                    ps = psp.tile([128, TC], F32, tag="b512")
                    for p in range(NPAIR):
                        nc.tensor.matmul(
                            ps[:], oT[p][:, ts(tt, 128)],
                            wo_sb[:, p * D + e * TC:p * D + (e + 1) * TC],
                            start=(p == 0), stop=(p == NPAIR - 1))
                    nc.vector.tensor_copy(ob[:, ts(e, TC)], ps[:])
                nc.sync.dma_start(out=out_d[ts(tt, 128), :], in_=ob[:])

    nc.compile()
    return nc


def kernel(**inputs):
    query = np.asarray(inputs["query"], np.float32)
    key = np.asarray(inputs["key"], np.float32)
    value = np.asarray(inputs["value"], np.float32)
    mask = np.asarray(inputs["mask"], bool)
    Wq, bq = np.asarray(inputs["Wq"], np.float32), np.asarray(inputs["bq"], np.float32)
    Wk, bk = np.asarray(inputs["Wk"], np.float32), np.asarray(inputs["bk"], np.float32)
    Wv, bv = np.asarray(inputs["Wv"], np.float32), np.asarray(inputs["bv"], np.float32)
    Wo, bo = np.asarray(inputs["Wo"], np.float32), np.asarray(inputs["bo"], np.float32)

    mode, cls, mixed = _classify_blocks(mask)
    global mixed_idx
    if mode == "general":
        mixed_idx = {blk: n for n, blk in enumerate(mixed)}
        n_mixed = len(mixed)
    else:
        mixed_idx, n_mixed = {}, 0

    key_sig = (mode, tuple(cls.ravel()) if cls is not None else None)
    if key_sig not in _cache:
        _cache[key_sig] = _build(mode, cls, n_mixed)
    nc = _cache[key_sig]

    in_maps = []
    xT = {}
    for b in range(B):
        xT[("xq", b)] = np.ascontiguousarray(query[b].T)
        xT[("xk", b)] = np.ascontiguousarray(key[b].T)
        xT[("xv", b)] = np.ascontiguousarray(value[b].T)
    for core in range(NCORE):
        b, g = core // 2, core % 2
        sl = slice(g * DG, (g + 1) * DG)
        im = {
            "xq": xT[("xq", b)], "xk": xT[("xk", b)], "xv": xT[("xv", b)],
            "wq": np.ascontiguousarray(Wq[sl, :].T),
            "wk": np.ascontiguousarray(Wk[sl, :].T),
            "wv": np.ascontiguousarray(Wv[sl, :].T),
            "wo": np.ascontiguousarray(Wo[:, sl].T).astype(ml_dtypes.bfloat16),
            "bq": np.ascontiguousarray(bq[sl].reshape(NPAIR, 128).T),
            "bk": np.ascontiguousarray(bk[sl].reshape(NPAIR, 128).T),
            "bv": np.ascontiguousarray(bv[sl])[None, :],
            "ones1": np.ones((1, 128), np.float32),
            "onesv": np.ones((128, H // 2), ml_dtypes.bfloat16),
        }
        if n_mixed:
            mm = np.empty((n_mixed, SB, TC), ml_dtypes.bfloat16)
            for n, (s_blk, i) in enumerate(mixed):
                blk = mask[b, i * TC:(i + 1) * TC, s_blk * SB:(s_blk + 1) * SB]
                mm[n] = (~blk.T).astype(np.float32)
            im["mmask"] = mm
        in_maps.append(im)

    r = run_bass_kernel_spmd(nc, in_maps, core_ids=list(range(NCORE)))
    last_result["exec_time_ns"] = r.exec_time_ns
    out = np.empty((B, T, D), np.float32)
    for b in range(B):
        out[b] = r.results[2 * b]["out"] + r.results[2 * b + 1]["out"]
    out += bo[None, None, :]
    return out
